# revision 15
# baseline (speedup 1.0000x reference)
"""Trainium2 Bass kernel for multi-head causal self-attention.

Problem: B=4, S=2048, D=768, H=12, DH=64 (fp32).
  Q = x @ W_Q + b_Q; K, V likewise
  scores = QK^T / sqrt(DH), causal mask, softmax
  out = (probs @ V) @ W_O + b_O

Sharding over 8 cores: core c -> batch b = c//2, head-half hh = c%2
(6 heads each). Fully local compute, no collectives; the two partial
outputs per batch are summed on the host during unshard (b_O also
added on host).

Device layout is "transposed" (sequence on the free dim):
  xT    [D, S]
  QT,KT [384, S]   hk on partitions (3 chunks of 128 = 2 heads each)
  V     [S, 768]   natural layout, 128 cols per head: [1 | 0*63 | V64]
                   so P^T@[1|0|V] accumulates softmax sums on PSUM
                   partition 0 and Z on partitions 64..127 in the same
                   accumulation (partition-aligned for the normalize)
  S^T   [keys, q]  scores transposed
  Z^T   [384, S]
  outT  [D, S]     host transposes back

Performance structure (vs the naive phased version):
  * Attention runs in 512-wide q "chains" per (head-pair, q-chunk),
    processed in anti-diagonal order so projection / output-projection
    matmuls are available as PE filler against the scalar engine's exp
    latency (the exp is the attention-phase bottleneck; PE gaps drop
    the tensor engine's DVFS p-state from 2.4GHz to 1.2GHz).
  * Score matmuls are K=64 and row-paired at PE tile positions (0,0)
    and (64,0) -> the two heads' score matmuls run concurrently.
  * One exp instruction per kt step covers both heads via a strided AP.
  * Softmax normalize: DVE copy [65,512] out of PSUM, DVE
    reciprocal_approx_fast on the sums row, gpsimd broadcast+multiply.
  * PE warmup matmuls during the input DMA phase keep the p-state ramp
    from eating into real work.
Softmax skips the max-subtraction (scores are ~N(0, 0.3)), which is
mathematically identical to the reference softmax.
"""

import numpy as np

import concourse.mybir as mybir
import concourse.tile as tile
from concourse import bacc, bass_utils

F32 = mybir.dt.float32
BF16 = mybir.dt.bfloat16

B, S, D, H, DH = 4, 2048, 768, 12, 64
HL = 6                # heads per core
HK = HL * DH          # 384
NPAIR = HL // 2       # 3 head pairs (128 partitions each)
P = 128
NDT = D // P          # 6 d-tiles
NST = S // P          # 16 key tiles
QW = 512              # q chain width (one PSUM bank)
NQC = S // QW         # 4 chains per head pair
VW = DH + 1           # 65 = V cols + ones col
VTW = 128             # V block stride per head: [1 | 0*63 | V64]
SCALE = 1.0 / 8.0     # 1/sqrt(DH)

# interleaver cost model (ns)
PEC = 1e9 / 2.4e9
ACTC = 1e9 / 1.2e9
ACT_OVH = 195.0

# anti-diagonal chain order: spreads deep (exp-heavy) chains across
# the kernel so projection/output-projection filler is available
CHAIN_ORDER = [(0, 0), (0, 1), (1, 0), (0, 2), (1, 1), (2, 0),
               (0, 3), (1, 2), (2, 1), (1, 3), (2, 2), (2, 3)]

N_WARMUP = 10


def _np_in(a):
    import ml_dtypes
    return np.ascontiguousarray(a, dtype=np.float32).astype(ml_dtypes.bfloat16)


def build_nc():
    nc = bacc.Bacc("TRN2", target_bir_lowering=False, debug=False, num_devices=8)

    xT = nc.dram_tensor("xT", [D, S], BF16, kind="ExternalInput").ap()
    wq = nc.dram_tensor("wq", [D, HK], BF16, kind="ExternalInput").ap()
    wk = nc.dram_tensor("wk", [D, HK], BF16, kind="ExternalInput").ap()
    wv = nc.dram_tensor("wv", [D, HK], BF16, kind="ExternalInput").ap()
    wo = nc.dram_tensor("wo", [HK, D], BF16, kind="ExternalInput").ap()
    bq = nc.dram_tensor("bq", [HK], F32, kind="ExternalInput").ap()
    bk = nc.dram_tensor("bk", [HK], F32, kind="ExternalInput").ap()
    bv = nc.dram_tensor("bv", [HK], BF16, kind="ExternalInput").ap()
    # additive causal mask for the diagonal 128x128 block, [key, q] layout
    mska = nc.dram_tensor("mska", [P, P], BF16, kind="ExternalInput").ap()
    iden = nc.dram_tensor("iden", [P, P], BF16, kind="ExternalInput").ap()
    out = nc.dram_tensor("out", [D, S], F32, kind="ExternalOutput").ap()

    EXP = mybir.ActivationFunctionType.Exp

    with tile.TileContext(nc) as tc:
        with (
            tc.tile_pool(name="big", bufs=1) as big,
            tc.tile_pool(name="wts", bufs=1) as wts,
            tc.tile_pool(name="vpool", bufs=1) as vpool,
            tc.tile_pool(name="small", bufs=1) as small,
            tc.tile_pool(name="pp", bufs=3) as pp,
            tc.tile_pool(name="ocp", bufs=4) as ocp_pool,
            tc.tile_pool(name="rrp", bufs=4) as rr_pool,
            tc.tile_pool(name="rbp", bufs=4) as rb_pool,
            tc.tile_pool(name="otp", bufs=4) as otp,
            tc.tile_pool(name="ps_s", bufs=2, space="PSUM") as ps_s,
            tc.tile_pool(name="ps_o", bufs=2, space="PSUM") as ps_o,
            tc.tile_pool(name="ps_p", bufs=2, space="PSUM") as ps_p,
        ):
            # ---- PE warmup: ramp the tensor engine p-state while input
            # DMAs are in flight (no data deps; junk matmuls on a memset
            # tile, result never read)
            warm_mm = small.tile([P, QW], BF16, tag="warmmm")
            nc.vector.memset(warm_mm, 0.5)
            warm_ps = ps_p.tile([P, QW], F32, tag="psp")
            for i in range(N_WARMUP):
                nc.tensor.matmul(
                    warm_ps, lhsT=warm_mm[:, 0:P], rhs=warm_mm,
                    start=True, stop=True,
                )

            # ---- constants / biases ------------------------------------
            mska_sb = small.tile([P, P], BF16, tag="mska")
            nc.gpsimd.dma_start(out=mska_sb, in_=mska)
            iden_sb = small.tile([P, P], BF16, tag="iden")
            nc.gpsimd.dma_start(out=iden_sb, in_=iden)
            # touch Exp once at t=0 so the ACT table load (~1.3us) overlaps
            # the input DMA phase
            warm_sb = small.tile([1, 8], F32, tag="warm")
            nc.vector.memset(warm_sb, 1.0)
            nc.scalar.activation(warm_sb, warm_sb, EXP)
            bq_sb = small.tile([P, NPAIR], F32, tag="bq")
            nc.gpsimd.dma_start(out=bq_sb, in_=bq.rearrange("(c p) -> p c", p=P))
            bk_sb = small.tile([P, NPAIR], F32, tag="bk")
            nc.gpsimd.dma_start(out=bk_sb, in_=bk.rearrange("(c p) -> p c", p=P))
            bv_sb = small.tile([1, HK], BF16, tag="bv")
            nc.gpsimd.dma_start(out=bv_sb, in_=bv.rearrange("(o k) -> o k", o=1))
            ones_row = small.tile([1, P], BF16, tag="ones")
            nc.vector.memset(ones_row, 1.0)

            # ---- input DMAs (issue order = need order) ------------------
            # x chunks qc-major so the first projections can start early
            xt = [big.tile([P, S], BF16, tag=f"xt{dt}", name=f"xt{dt}") for dt in range(NDT)]
            wq_sb = []
            wk_sb = []
            for dram, lst, nm in ((wq, wq_sb, "wq"), (wk, wk_sb, "wk")):
                for dt in range(NDT):
                    t = wts.tile([P, HK], BF16, tag=f"{nm}{dt}")
                    nc.sync.dma_start(out=t, in_=dram[dt * P:(dt + 1) * P, :])
                    lst.append(t)
            for qc in range(NQC):
                for dt in range(NDT):
                    eng = nc.scalar if dt % 2 == 0 else nc.gpsimd
                    eng.dma_start(
                        out=xt[dt][:, qc * QW:(qc + 1) * QW],
                        in_=xT[dt * P:(dt + 1) * P, qc * QW:(qc + 1) * QW],
                    )
            wv_sb = []
            for dt in range(NDT):
                t = wts.tile([P, HK], BF16, tag=f"wv{dt}")
                nc.sync.dma_start(out=t, in_=wv[dt * P:(dt + 1) * P, :])
                wv_sb.append(t)
            wo_sb = []
            for c in range(NPAIR):
                t = wts.tile([P, D], BF16, tag=f"wo{c}")
                nc.sync.dma_start(out=t, in_=wo[c * P:(c + 1) * P, :])
                wo_sb.append(t)

            # ---- persistent compute tiles -------------------------------
            QT = [big.tile([P, S], BF16, tag=f"qt{c}", name=f"qt{c}") for c in range(NPAIR)]
            KT = [big.tile([P, S], BF16, tag=f"kt{c}", name=f"kt{c}") for c in range(NPAIR)]
            ZT = [big.tile([P, S], BF16, tag=f"zt{c}", name=f"zt{c}") for c in range(NPAIR)]
            Vt = []
            for st in range(NST):
                t = vpool.tile([P, HL * VTW], BF16, tag=f"v{st}", name=f"v{st}")
                vv = t.rearrange("p (h c) -> p h c", c=VTW)
                nc.vector.memset(vv[:, :, 0:1], 1.0)
                nc.vector.memset(vv[:, :, 1:DH], 0.0)
                Vt.append(t)

            # ---- interleaver state --------------------------------------
            state = {"pe": 0.0, "act": 0.0, "copy_alt": 0}
            fillers = []      # list of (emit_fn, pe_cost)

            def copy_engine():
                # alternate proj-copy engine between ACT and DVE to split
                # the PSUM-egress load
                state["copy_alt"] += 1
                return nc.scalar if state["copy_alt"] % 2 == 0 else nc.vector

            def emit_qk(which, pr, qc):
                w_sb, b_sb, dst = (
                    (wq_sb, bq_sb, QT) if which == "q" else (wk_sb, bk_sb, KT)
                )
                pt = ps_p.tile([P, QW], F32, tag="psp")
                for dt in range(NDT):
                    nc.tensor.matmul(
                        pt,
                        lhsT=w_sb[dt][:, pr * P:(pr + 1) * P],
                        rhs=xt[dt][:, qc * QW:(qc + 1) * QW],
                        start=(dt == 0),
                        stop=(dt == NDT - 1),
                    )
                eng = copy_engine()
                dst_ap = dst[pr][:, qc * QW:(qc + 1) * QW]
                if eng is nc.scalar:
                    nc.scalar.add(dst_ap, pt, b_sb[:, pr:pr + 1])
                else:
                    nc.vector.tensor_scalar_add(dst_ap, pt, b_sb[:, pr:pr + 1])
                state["pe"] += 6 * QW * PEC

            def emit_v(st):
                pt = ps_p.tile([P, QW], F32, tag="psp")
                for dt in range(NDT):
                    nc.tensor.matmul(
                        pt[:, 0:HK],
                        lhsT=xt[dt][:, st * P:(st + 1) * P],
                        rhs=wv_sb[dt],
                        start=(dt == 0),
                        stop=False,
                    )
                nc.tensor.matmul(
                    pt[:, 0:HK],
                    lhsT=ones_row,
                    rhs=bv_sb,
                    start=False,
                    stop=True,
                )
                vv = Vt[st].rearrange("p (h c) -> p h c", c=VTW)
                eng = copy_engine()
                eng_copy = nc.scalar.copy if eng is nc.scalar else nc.vector.tensor_copy
                eng_copy(
                    vv[:, :, DH:VTW],
                    pt[:, 0:HK].rearrange("p (h c) -> p h c", c=DH),
                )
                state["pe"] += 7 * HK * PEC

            def emit_oproj(qc, dt):
                pt = ps_p.tile([P, QW], F32, tag="psp")
                for c in range(NPAIR):
                    nc.tensor.matmul(
                        pt,
                        lhsT=wo_sb[c][:, dt * P:(dt + 1) * P],
                        rhs=ZT[c][:, qc * QW:(qc + 1) * QW],
                        start=(c == 0),
                        stop=(c == NPAIR - 1),
                    )
                osb = otp.tile([P, QW], F32, tag="ot")
                eng = copy_engine()
                eng_copy = nc.scalar.copy if eng is nc.scalar else nc.vector.tensor_copy
                eng_copy(osb, pt)
                nc.sync.dma_start(
                    out=out[dt * P:(dt + 1) * P, qc * QW:(qc + 1) * QW],
                    in_=osb,
                )
                state["pe"] += NPAIR * QW * PEC

            pending = []      # (ready_step, emit_fn) gated fillers
            state["step"] = 0

            def pull_fillers():
                while pending and pending[0][0] <= state["step"]:
                    fillers.append(pending.pop(0)[1])
                while fillers and state["act"] > state["pe"]:
                    fn = fillers.pop(0)
                    fn()

            # dependency bookkeeping: JIT-emit projections a chain needs
            qk_done = set()
            v_done = set()

            def need_qk(pr, qc):
                for q in range(qc + 1):
                    for which in ("q", "k"):
                        if (which, pr, q) not in qk_done:
                            qk_done.add((which, pr, q))
                            emit_qk(which, pr, q)

            def need_v(kt_max):
                for st in range(kt_max + 1):
                    if st not in v_done:
                        v_done.add(st)
                        emit_v(st)

            # ---- attention chains ---------------------------------------
            def emit_chain(pr, qc):
                nkt = 4 * qc + 4
                need_qk(pr, qc)
                need_v(nkt - 1)
                q0 = qc * QW
                O_ab = [ps_o.tile([P, QW], F32, tag="o", name=f"o{i}") for i in range(2)]
                for kt in range(nkt):
                    o = max(0, P * kt - q0)
                    diag = P * kt >= q0
                    w = QW - o
                    sp = ps_s.tile([P, 2 * QW], F32, tag="s")
                    for hh in range(2):
                        lo = hh * DH
                        nc.tensor.matmul(
                            sp[:, hh * QW + o:hh * QW + QW],
                            lhsT=KT[pr][lo:lo + DH, kt * P:(kt + 1) * P],
                            rhs=QT[pr][lo:lo + DH, q0 + o:q0 + QW],
                            start=True,
                            stop=not diag,
                        )
                    if diag:
                        for hh in range(2):
                            nc.tensor.matmul(
                                sp[:, hh * QW + o:hh * QW + o + P],
                                lhsT=iden_sb,
                                rhs=mska_sb,
                                start=False,
                                stop=True,
                            )
                    # one contiguous exp for both heads; the junk columns
                    # [QW:QW+o) in the middle are never read downstream
                    pt = pp.tile([P, 2 * QW], BF16, tag="p")
                    nc.scalar.activation(
                        pt[:, o:2 * QW], sp[:, o:2 * QW], EXP, scale=SCALE
                    )
                    for hh in range(2):
                        h0 = (2 * pr + hh) * VTW
                        nc.tensor.matmul(
                            O_ab[hh][:, o:QW],
                            lhsT=Vt[kt][:, h0:h0 + VTW],
                            rhs=pt[:, hh * QW + o:hh * QW + QW],
                            start=(kt == 0),
                            stop=(kt == nkt - 1),
                        )
                    state["pe"] += (3 * w + (2 * P if diag else 0)) * PEC
                    state["act"] += (2 * QW - o) * ACTC + ACT_OVH
                    state["step"] += 1
                    pull_fillers()
                # normalize: ZT = O[0:64] * 1/sums, sums = O[64]
                for hh in range(2):
                    lo = hh * DH
                    # copy O out of PSUM first so the bank frees before the
                    # next chain's first PV needs it
                    oc = ocp_pool.tile([P, QW], F32, tag="ocp")
                    nc.vector.tensor_copy(oc, O_ab[hh])
                    rr = rr_pool.tile([1, QW], F32, tag="rr")
                    nc.vector.reciprocal_approx_fast(out=rr, in_=oc[0:1, :])
                    rb = rb_pool.tile([P, QW], F32, tag="rb")
                    nc.gpsimd.partition_broadcast(rb, rr)
                    nc.vector.tensor_mul(
                        ZT[pr][lo:lo + DH, q0:q0 + QW], oc[DH:P, :],
                        rb[DH:P, :]
                    )

            # o-proj units for qc become pullable a few attention steps
            # after chain (2, qc)'s normalize was emitted
            for pr, qc in CHAIN_ORDER:
                emit_chain(pr, qc)
                if pr == 2:
                    for dt in range(NDT):
                        pending.append(
                            (state["step"] + 4,
                             lambda qc=qc, dt=dt: emit_oproj(qc, dt))
                        )

            # drain remaining fillers (final o-proj blocks)
            for _, fn in pending:
                fillers.append(fn)
            for fn in fillers:
                fn()

    nc.compile()
    return nc


_NC_CACHE = {}


def _get_nc():
    if "nc" not in _NC_CACHE:
        _NC_CACHE["nc"] = build_nc()
    return _NC_CACHE["nc"]


def make_in_maps(x, W_Q, W_K, W_V, W_O, b_Q, b_K, b_V, b_O):
    mask_add = np.tril(np.full((P, P), -1e4, np.float32), k=-1)
    identity = np.eye(P, dtype=np.float32)
    in_maps = []
    for c in range(8):
        b, hh = divmod(c, 2)
        hs = slice(HL * hh, HL * hh + HL)
        in_maps.append({
            "xT": _np_in(np.asarray(x[b]).T),
            "wq": _np_in(np.asarray(W_Q[hs]).transpose(1, 0, 2).reshape(D, HK)),
            "wk": _np_in(np.asarray(W_K[hs]).transpose(1, 0, 2).reshape(D, HK)),
            "wv": _np_in(np.asarray(W_V[hs]).transpose(1, 0, 2).reshape(D, HK)),
            "wo": _np_in(np.asarray(W_O[hs]).reshape(HK, D)),
            "bq": np.ascontiguousarray(np.asarray(b_Q[hs]).reshape(HK), np.float32),
            "bk": np.ascontiguousarray(np.asarray(b_K[hs]).reshape(HK), np.float32),
            "bv": _np_in(np.asarray(b_V[hs]).reshape(HK)),
            "mska": _np_in(mask_add),
            "iden": _np_in(identity),
        })
    return in_maps


def run(inputs, trace=False):
    nc = _get_nc()
    in_maps = make_in_maps(**inputs)
    res = bass_utils.run_bass_kernel_spmd(
        nc, in_maps, core_ids=list(range(8)), trace=trace,
        **({"trace_cores": [0]} if trace else {}),
    )
    outs = [r["out"] for r in res.results]
    bo = np.ascontiguousarray(np.asarray(inputs["b_O"]), np.float32)
    full = np.empty((B, S, D), np.float32)
    for b in range(B):
        full[b] = (outs[2 * b] + outs[2 * b + 1]).T + bo
    return full, res


def kernel(**inputs):
    full, _ = run(inputs)
    return full


# revision 16
# speedup vs baseline: 1.1568x; 1.1568x over previous
"""Trainium2 Bass kernel for multi-head causal self-attention.

Problem: B=4, S=2048, D=768, H=12, DH=64 (fp32).
  Q = x @ W_Q + b_Q; K, V likewise
  scores = QK^T / sqrt(DH), causal mask, softmax
  out = (probs @ V) @ W_O + b_O

Sharding over 8 cores: core c -> batch b = c//2, head-half hh = c%2
(6 heads each). Fully local compute, no collectives; the two partial
outputs per batch are summed on the host during unshard (b_O also
added on host).

Device layout is "transposed" (sequence on the free dim):
  xT    [D, S]
  QT,KT [384, S]   hk on partitions (3 chunks of 128 = 2 heads each)
  V     [S, 768]   natural layout, 128 cols per head: [1 | 0*63 | V64]
                   so P^T@[1|0|V] accumulates softmax sums on PSUM
                   partition 0 and Z on partitions 64..127 in the same
                   accumulation (partition-aligned for the normalize)
  S^T   [keys, q]  scores transposed
  Z^T   [384, S]
  outT  [D, S]     host transposes back

Performance structure (vs the naive phased version):
  * Attention runs in 512-wide q "chains" per (head-pair, q-chunk),
    processed in anti-diagonal order so projection / output-projection
    matmuls are available as PE filler against the scalar engine's exp
    latency (the exp is the attention-phase bottleneck; PE gaps drop
    the tensor engine's DVFS p-state from 2.4GHz to 1.2GHz).
  * Score matmuls are K=64 and row-paired at PE tile positions (0,0)
    and (64,0) -> the two heads' score matmuls run concurrently.
  * One exp instruction per kt step covers both heads via a strided AP.
  * Softmax normalize: DVE copy [65,512] out of PSUM, DVE
    reciprocal_approx_fast on the sums row, gpsimd broadcast+multiply.
  * PE warmup matmuls during the input DMA phase keep the p-state ramp
    from eating into real work.
Softmax skips the max-subtraction (scores are ~N(0, 0.3)), which is
mathematically identical to the reference softmax.
"""

import numpy as np

import concourse.mybir as mybir
import concourse.tile as tile
from concourse import bacc, bass_utils

F32 = mybir.dt.float32
BF16 = mybir.dt.bfloat16

B, S, D, H, DH = 4, 2048, 768, 12, 64
HL = 6                # heads per core
HK = HL * DH          # 384
NPAIR = HL // 2       # 3 head pairs (128 partitions each)
P = 128
NDT = D // P          # 6 d-tiles
NST = S // P          # 16 key tiles
QW = 512              # q chain width (one PSUM bank)
NQC = S // QW         # 4 chains per head pair
VW = DH + 1           # 65 = V cols + ones col
VTW = 128             # V block stride per head: [1 | 0*63 | V64]
SCALE = 1.0 / 8.0     # 1/sqrt(DH)

# interleaver cost model (ns)
PEC = 1e9 / 2.4e9
ACTC = 1e9 / 1.2e9
ACT_OVH = 195.0

# anti-diagonal chain order: spreads deep (exp-heavy) chains across
# the kernel so projection/output-projection filler is available
CHAIN_ORDER = [(0, 0), (0, 1), (1, 0), (0, 2), (1, 1), (2, 0),
               (0, 3), (1, 2), (2, 1), (1, 3), (2, 2), (2, 3)]

N_WARMUP = 10


def _np_in(a):
    import ml_dtypes
    return np.ascontiguousarray(a, dtype=np.float32).astype(ml_dtypes.bfloat16)


def build_nc():
    nc = bacc.Bacc("TRN2", target_bir_lowering=False, debug=False, num_devices=8)

    xT = nc.dram_tensor("xT", [D, S], BF16, kind="ExternalInput").ap()
    wq = nc.dram_tensor("wq", [D, HK], BF16, kind="ExternalInput").ap()
    wk = nc.dram_tensor("wk", [D, HK], BF16, kind="ExternalInput").ap()
    wv = nc.dram_tensor("wv", [D, HK], BF16, kind="ExternalInput").ap()
    wo = nc.dram_tensor("wo", [HK, D], BF16, kind="ExternalInput").ap()
    bq = nc.dram_tensor("bq", [HK], F32, kind="ExternalInput").ap()
    bk = nc.dram_tensor("bk", [HK], F32, kind="ExternalInput").ap()
    bv = nc.dram_tensor("bv", [HK], BF16, kind="ExternalInput").ap()
    # additive causal mask for the diagonal 128x128 block, [key, q] layout
    mska = nc.dram_tensor("mska", [P, P], BF16, kind="ExternalInput").ap()
    iden = nc.dram_tensor("iden", [P, P], BF16, kind="ExternalInput").ap()
    out = nc.dram_tensor("out", [D, S], F32, kind="ExternalOutput").ap()

    EXP = mybir.ActivationFunctionType.Exp

    with tile.TileContext(nc) as tc:
        with (
            tc.tile_pool(name="big", bufs=1) as big,
            tc.tile_pool(name="wts", bufs=1) as wts,
            tc.tile_pool(name="vpool", bufs=1) as vpool,
            tc.tile_pool(name="small", bufs=1) as small,
            tc.tile_pool(name="pp", bufs=3) as pp,
            tc.tile_pool(name="ocp", bufs=4) as ocp_pool,
            tc.tile_pool(name="rrp", bufs=4) as rr_pool,
            tc.tile_pool(name="rbp", bufs=4) as rb_pool,
            tc.tile_pool(name="otp", bufs=4) as otp,
            tc.tile_pool(name="ps_s", bufs=2, space="PSUM") as ps_s,
            tc.tile_pool(name="ps_o", bufs=2, space="PSUM") as ps_o,
            tc.tile_pool(name="ps_p", bufs=2, space="PSUM") as ps_p,
        ):
            # ---- PE warmup: ramp the tensor engine p-state while input
            # DMAs are in flight (no data deps; junk matmuls on a memset
            # tile, result never read)
            warm_mm = small.tile([P, QW], BF16, tag="warmmm")
            nc.vector.memset(warm_mm, 0.5)
            warm_ps = ps_p.tile([P, QW], F32, tag="psp")
            for i in range(N_WARMUP):
                nc.tensor.matmul(
                    warm_ps, lhsT=warm_mm[:, 0:P], rhs=warm_mm,
                    start=True, stop=True,
                )

            # ---- constants / biases ------------------------------------
            mska_sb = small.tile([P, P], BF16, tag="mska")
            nc.gpsimd.dma_start(out=mska_sb, in_=mska)
            iden_sb = small.tile([P, P], BF16, tag="iden")
            nc.gpsimd.dma_start(out=iden_sb, in_=iden)
            # touch Exp once at t=0 so the ACT table load (~1.3us) overlaps
            # the input DMA phase
            warm_sb = small.tile([1, 8], F32, tag="warm")
            nc.vector.memset(warm_sb, 1.0)
            nc.scalar.activation(warm_sb, warm_sb, EXP)
            bq_sb = small.tile([P, NPAIR], F32, tag="bq")
            nc.gpsimd.dma_start(out=bq_sb, in_=bq.rearrange("(c p) -> p c", p=P))
            bk_sb = small.tile([P, NPAIR], F32, tag="bk")
            nc.gpsimd.dma_start(out=bk_sb, in_=bk.rearrange("(c p) -> p c", p=P))
            bv_sb = small.tile([1, HK], BF16, tag="bv")
            nc.gpsimd.dma_start(out=bv_sb, in_=bv.rearrange("(o k) -> o k", o=1))
            ones_row = small.tile([1, P], BF16, tag="ones")
            nc.vector.memset(ones_row, 1.0)

            # ---- input DMAs (issue order = need order) ------------------
            # x chunks qc-major so the first projections can start early
            xt = [big.tile([P, S], BF16, tag=f"xt{dt}", name=f"xt{dt}") for dt in range(NDT)]
            wq_sb = []
            wk_sb = []
            for dram, lst, nm in ((wq, wq_sb, "wq"), (wk, wk_sb, "wk")):
                for dt in range(NDT):
                    t = wts.tile([P, HK], BF16, tag=f"{nm}{dt}")
                    nc.sync.dma_start(out=t, in_=dram[dt * P:(dt + 1) * P, :])
                    lst.append(t)
            for qc in range(NQC):
                for dt in range(NDT):
                    eng = nc.scalar if dt % 2 == 0 else nc.gpsimd
                    eng.dma_start(
                        out=xt[dt][:, qc * QW:(qc + 1) * QW],
                        in_=xT[dt * P:(dt + 1) * P, qc * QW:(qc + 1) * QW],
                    )
            wv_sb = []
            for dt in range(NDT):
                t = wts.tile([P, HK], BF16, tag=f"wv{dt}")
                nc.sync.dma_start(out=t, in_=wv[dt * P:(dt + 1) * P, :])
                wv_sb.append(t)
            wo_sb = []
            for c in range(NPAIR):
                t = wts.tile([P, D], BF16, tag=f"wo{c}")
                nc.sync.dma_start(out=t, in_=wo[c * P:(c + 1) * P, :])
                wo_sb.append(t)

            # ---- persistent compute tiles -------------------------------
            QT = [big.tile([P, S], BF16, tag=f"qt{c}", name=f"qt{c}") for c in range(NPAIR)]
            KT = [big.tile([P, S], BF16, tag=f"kt{c}", name=f"kt{c}") for c in range(NPAIR)]
            ZT = [big.tile([P, S], BF16, tag=f"zt{c}", name=f"zt{c}") for c in range(NPAIR)]
            Vt = []
            for st in range(NST):
                t = vpool.tile([P, HL * VTW], BF16, tag=f"v{st}", name=f"v{st}")
                vv = t.rearrange("p (h c) -> p h c", c=VTW)
                nc.vector.memset(vv[:, :, 0:1], 1.0)
                nc.vector.memset(vv[:, :, 1:DH], 0.0)
                Vt.append(t)

            # ---- interleaver state --------------------------------------
            state = {"pe": 0.0, "act": 0.0, "copy_alt": 0}
            fillers = []      # list of (emit_fn, pe_cost)

            def copy_engine():
                # alternate proj-copy engine between ACT and DVE to split
                # the PSUM-egress load
                state["copy_alt"] += 1
                return nc.scalar if state["copy_alt"] % 2 == 0 else nc.vector

            def emit_qk(which, pr, qc):
                w_sb, b_sb, dst = (
                    (wq_sb, bq_sb, QT) if which == "q" else (wk_sb, bk_sb, KT)
                )
                pt = ps_p.tile([P, QW], F32, tag="psp")
                for dt in range(NDT):
                    nc.tensor.matmul(
                        pt,
                        lhsT=w_sb[dt][:, pr * P:(pr + 1) * P],
                        rhs=xt[dt][:, qc * QW:(qc + 1) * QW],
                        start=(dt == 0),
                        stop=(dt == NDT - 1),
                    )
                eng = copy_engine()
                dst_ap = dst[pr][:, qc * QW:(qc + 1) * QW]
                if eng is nc.scalar:
                    nc.scalar.add(dst_ap, pt, b_sb[:, pr:pr + 1])
                else:
                    nc.vector.tensor_scalar_add(dst_ap, pt, b_sb[:, pr:pr + 1])
                state["pe"] += 6 * QW * PEC

            def emit_v(st):
                pt = ps_p.tile([P, QW], F32, tag="psp")
                for dt in range(NDT):
                    nc.tensor.matmul(
                        pt[:, 0:HK],
                        lhsT=xt[dt][:, st * P:(st + 1) * P],
                        rhs=wv_sb[dt],
                        start=(dt == 0),
                        stop=False,
                    )
                nc.tensor.matmul(
                    pt[:, 0:HK],
                    lhsT=ones_row,
                    rhs=bv_sb,
                    start=False,
                    stop=True,
                )
                vv = Vt[st].rearrange("p (h c) -> p h c", c=VTW)
                eng = copy_engine()
                eng_copy = nc.scalar.copy if eng is nc.scalar else nc.vector.tensor_copy
                eng_copy(
                    vv[:, :, DH:VTW],
                    pt[:, 0:HK].rearrange("p (h c) -> p h c", c=DH),
                )
                state["pe"] += 7 * HK * PEC

            def emit_oproj(qc, dt):
                pt = ps_p.tile([P, QW], F32, tag="psp")
                for c in range(NPAIR):
                    nc.tensor.matmul(
                        pt,
                        lhsT=wo_sb[c][:, dt * P:(dt + 1) * P],
                        rhs=ZT[c][:, qc * QW:(qc + 1) * QW],
                        start=(c == 0),
                        stop=(c == NPAIR - 1),
                    )
                osb = otp.tile([P, QW], F32, tag="ot")
                eng = copy_engine()
                eng_copy = nc.scalar.copy if eng is nc.scalar else nc.vector.tensor_copy
                eng_copy(osb, pt)
                nc.sync.dma_start(
                    out=out[dt * P:(dt + 1) * P, qc * QW:(qc + 1) * QW],
                    in_=osb,
                )
                state["pe"] += NPAIR * QW * PEC

            pending = []      # (ready_step, emit_fn) gated fillers
            state["step"] = 0

            def pull_fillers():
                while pending and pending[0][0] <= state["step"]:
                    fillers.append(pending.pop(0)[1])
                while fillers and state["act"] > state["pe"]:
                    fn = fillers.pop(0)
                    fn()

            # dependency bookkeeping: JIT-emit projections a chain needs
            qk_done = set()
            v_done = set()

            def need_qk(pr, qc):
                for q in range(qc + 1):
                    for which in ("q", "k"):
                        if (which, pr, q) not in qk_done:
                            qk_done.add((which, pr, q))
                            emit_qk(which, pr, q)

            def need_v(kt_max):
                for st in range(kt_max + 1):
                    if st not in v_done:
                        v_done.add(st)
                        emit_v(st)

            # ---- attention chains ---------------------------------------
            def emit_chain(pr, qc):
                nkt = 4 * qc + 4
                need_qk(pr, qc)
                need_v(nkt - 1)
                q0 = qc * QW
                O_ab = [ps_o.tile([P, QW], F32, tag="o", name=f"o{i}") for i in range(2)]
                for kt in range(nkt):
                    o = max(0, P * kt - q0)
                    diag = P * kt >= q0
                    w = QW - o
                    sp = ps_s.tile([P, 2 * QW], F32, tag="s")
                    for hh in range(2):
                        lo = hh * DH
                        nc.tensor.matmul(
                            sp[:, hh * QW + o:hh * QW + QW],
                            lhsT=KT[pr][lo:lo + DH, kt * P:(kt + 1) * P],
                            rhs=QT[pr][lo:lo + DH, q0 + o:q0 + QW],
                            start=True,
                            stop=not diag,
                        )
                    if diag:
                        for hh in range(2):
                            nc.tensor.matmul(
                                sp[:, hh * QW + o:hh * QW + o + P],
                                lhsT=iden_sb,
                                rhs=mska_sb,
                                start=False,
                                stop=True,
                            )
                    # one contiguous exp for both heads; the junk columns
                    # [QW:QW+o) in the middle are never read downstream
                    pt = pp.tile([P, 2 * QW], BF16, tag="p")
                    nc.scalar.activation(
                        pt[:, o:2 * QW], sp[:, o:2 * QW], EXP, scale=SCALE
                    )
                    for hh in range(2):
                        h0 = (2 * pr + hh) * VTW
                        nc.tensor.matmul(
                            O_ab[hh][:, o:QW],
                            lhsT=Vt[kt][:, h0:h0 + VTW],
                            rhs=pt[:, hh * QW + o:hh * QW + QW],
                            start=(kt == 0),
                            stop=(kt == nkt - 1),
                        )
                    state["pe"] += (3 * w + (2 * P if diag else 0)) * PEC
                    state["act"] += (2 * QW - o) * ACTC + ACT_OVH
                    state["step"] += 1
                    pull_fillers()
                # normalize: ZT = O[0:64] * 1/sums, sums = O[64]
                for hh in range(2):
                    lo = hh * DH
                    rr = rr_pool.tile([1, QW], F32, tag="rr")
                    nc.vector.reciprocal_approx_fast(out=rr, in_=O_ab[hh][0:1, :])
                    rb = rb_pool.tile([P, QW], F32, tag="rb")
                    nc.gpsimd.partition_broadcast(rb, rr)
                    nc.vector.tensor_mul(
                        ZT[pr][lo:lo + DH, q0:q0 + QW], O_ab[hh][DH:P, :],
                        rb[DH:P, :]
                    )

            # o-proj units for qc become pullable a few attention steps
            # after chain (2, qc)'s normalize was emitted
            for pr, qc in CHAIN_ORDER:
                emit_chain(pr, qc)
                if pr == 2:
                    for dt in range(NDT):
                        pending.append(
                            (state["step"] + 4,
                             lambda qc=qc, dt=dt: emit_oproj(qc, dt))
                        )

            # drain remaining fillers (final o-proj blocks)
            for _, fn in pending:
                fillers.append(fn)
            for fn in fillers:
                fn()

    nc.compile()
    return nc


_NC_CACHE = {}


def _get_nc():
    if "nc" not in _NC_CACHE:
        _NC_CACHE["nc"] = build_nc()
    return _NC_CACHE["nc"]


def make_in_maps(x, W_Q, W_K, W_V, W_O, b_Q, b_K, b_V, b_O):
    mask_add = np.tril(np.full((P, P), -1e4, np.float32), k=-1)
    identity = np.eye(P, dtype=np.float32)
    in_maps = []
    for c in range(8):
        b, hh = divmod(c, 2)
        hs = slice(HL * hh, HL * hh + HL)
        in_maps.append({
            "xT": _np_in(np.asarray(x[b]).T),
            "wq": _np_in(np.asarray(W_Q[hs]).transpose(1, 0, 2).reshape(D, HK)),
            "wk": _np_in(np.asarray(W_K[hs]).transpose(1, 0, 2).reshape(D, HK)),
            "wv": _np_in(np.asarray(W_V[hs]).transpose(1, 0, 2).reshape(D, HK)),
            "wo": _np_in(np.asarray(W_O[hs]).reshape(HK, D)),
            "bq": np.ascontiguousarray(np.asarray(b_Q[hs]).reshape(HK), np.float32),
            "bk": np.ascontiguousarray(np.asarray(b_K[hs]).reshape(HK), np.float32),
            "bv": _np_in(np.asarray(b_V[hs]).reshape(HK)),
            "mska": _np_in(mask_add),
            "iden": _np_in(identity),
        })
    return in_maps


def run(inputs, trace=False):
    nc = _get_nc()
    in_maps = make_in_maps(**inputs)
    res = bass_utils.run_bass_kernel_spmd(
        nc, in_maps, core_ids=list(range(8)), trace=trace,
        **({"trace_cores": [0]} if trace else {}),
    )
    outs = [r["out"] for r in res.results]
    bo = np.ascontiguousarray(np.asarray(inputs["b_O"]), np.float32)
    full = np.empty((B, S, D), np.float32)
    for b in range(B):
        full[b] = (outs[2 * b] + outs[2 * b + 1]).T + bo
    return full, res


def kernel(**inputs):
    full, _ = run(inputs)
    return full


# revision 18
# speedup vs baseline: 1.1656x; 1.0076x over previous
"""Trainium2 Bass kernel for multi-head causal self-attention.

Problem: B=4, S=2048, D=768, H=12, DH=64 (fp32).
  Q = x @ W_Q + b_Q; K, V likewise
  scores = QK^T / sqrt(DH), causal mask, softmax
  out = (probs @ V) @ W_O + b_O

Sharding over 8 cores: core c -> batch b = c//2, head-half hh = c%2
(6 heads each). Fully local compute, no collectives; the two partial
outputs per batch are summed on the host during unshard (b_O also
added on host).

Device layout is "transposed" (sequence on the free dim):
  xT    [D, S]
  QT,KT [384, S]   hk on partitions (3 chunks of 128 = 2 heads each)
  V     [S, 768]   natural layout, 128 cols per head: [1 | 0*63 | V64]
                   so P^T@[1|0|V] accumulates softmax sums on PSUM
                   partition 0 and Z on partitions 64..127 in the same
                   accumulation (partition-aligned for the normalize)
  S^T   [keys, q]  scores transposed
  Z^T   [384, S]
  outT  [D, S]     host transposes back

Performance structure (vs the naive phased version):
  * Attention runs in 512-wide q "chains" per (head-pair, q-chunk),
    processed in anti-diagonal order so projection / output-projection
    matmuls are available as PE filler against the scalar engine's exp
    latency (the exp is the attention-phase bottleneck; PE gaps drop
    the tensor engine's DVFS p-state from 2.4GHz to 1.2GHz).
  * Score matmuls are K=64 and row-paired at PE tile positions (0,0)
    and (64,0) -> the two heads' score matmuls run concurrently.
  * One exp instruction per kt step covers both heads via a strided AP.
  * Softmax normalize: DVE copy [65,512] out of PSUM, DVE
    reciprocal_approx_fast on the sums row, gpsimd broadcast+multiply.
  * PE warmup matmuls during the input DMA phase keep the p-state ramp
    from eating into real work.
Softmax skips the max-subtraction (scores are ~N(0, 0.3)), which is
mathematically identical to the reference softmax.
"""

import numpy as np

import concourse.mybir as mybir
import concourse.tile as tile
from concourse import bacc, bass_utils

F32 = mybir.dt.float32
BF16 = mybir.dt.bfloat16

B, S, D, H, DH = 4, 2048, 768, 12, 64
HL = 6                # heads per core
HK = HL * DH          # 384
NPAIR = HL // 2       # 3 head pairs (128 partitions each)
P = 128
NDT = D // P          # 6 d-tiles
NST = S // P          # 16 key tiles
QW = 512              # q chain width (one PSUM bank)
NQC = S // QW         # 4 chains per head pair
VW = DH + 1           # 65 = V cols + ones col
VTW = 128             # V block stride per head: [1 | 0*63 | V64]
SCALE = 1.0 / 8.0     # 1/sqrt(DH)

# interleaver cost model (ns)
PEC = 1e9 / 2.4e9
ACTC = 1e9 / 1.2e9
ACT_OVH = 195.0

# anti-diagonal chain order: spreads deep (exp-heavy) chains across
# the kernel so projection/output-projection filler is available
CHAIN_ORDER = [(0, 0), (0, 1), (1, 0), (0, 2), (1, 1), (2, 0),
               (0, 3), (1, 2), (2, 1), (1, 3), (2, 2), (2, 3)]

N_WARMUP = 10


def _np_in(a):
    import ml_dtypes
    return np.ascontiguousarray(a, dtype=np.float32).astype(ml_dtypes.bfloat16)


def build_nc():
    nc = bacc.Bacc("TRN2", target_bir_lowering=False, debug=False, num_devices=8)

    xT = nc.dram_tensor("xT", [D, S], BF16, kind="ExternalInput").ap()
    wq = nc.dram_tensor("wq", [D, HK], BF16, kind="ExternalInput").ap()
    wk = nc.dram_tensor("wk", [D, HK], BF16, kind="ExternalInput").ap()
    wv = nc.dram_tensor("wv", [D, HK], BF16, kind="ExternalInput").ap()
    wo = nc.dram_tensor("wo", [HK, D], BF16, kind="ExternalInput").ap()
    bq = nc.dram_tensor("bq", [HK], F32, kind="ExternalInput").ap()
    bk = nc.dram_tensor("bk", [HK], F32, kind="ExternalInput").ap()
    bv = nc.dram_tensor("bv", [HK], BF16, kind="ExternalInput").ap()
    # additive causal mask for the diagonal 128x128 block, [key, q] layout
    mska = nc.dram_tensor("mska", [P, P], BF16, kind="ExternalInput").ap()
    iden = nc.dram_tensor("iden", [P, P], BF16, kind="ExternalInput").ap()
    out = nc.dram_tensor("out", [D, S], F32, kind="ExternalOutput").ap()

    EXP = mybir.ActivationFunctionType.Exp

    with tile.TileContext(nc) as tc:
        with (
            tc.tile_pool(name="big", bufs=1) as big,
            tc.tile_pool(name="wts", bufs=1) as wts,
            tc.tile_pool(name="vpool", bufs=1) as vpool,
            tc.tile_pool(name="small", bufs=1) as small,
            tc.tile_pool(name="pp", bufs=3) as pp,
            tc.tile_pool(name="ocp", bufs=4) as ocp_pool,
            tc.tile_pool(name="rrp", bufs=4) as rr_pool,
            tc.tile_pool(name="rbp", bufs=4) as rb_pool,
            tc.tile_pool(name="otp", bufs=4) as otp,
            tc.tile_pool(name="ps_s", bufs=2, space="PSUM") as ps_s,
            tc.tile_pool(name="ps_o", bufs=2, space="PSUM") as ps_o,
            tc.tile_pool(name="ps_p", bufs=2, space="PSUM") as ps_p,
        ):
            # ---- PE warmup: ramp the tensor engine p-state while input
            # DMAs are in flight (no data deps; junk matmuls on a memset
            # tile, result never read)
            warm_mm = small.tile([P, QW], BF16, tag="warmmm")
            nc.vector.memset(warm_mm, 0.5)
            warm_ps = ps_p.tile([P, QW], F32, tag="psp")
            for i in range(N_WARMUP):
                nc.tensor.matmul(
                    warm_ps, lhsT=warm_mm[:, 0:P], rhs=warm_mm,
                    start=True, stop=True,
                )

            # ---- constants / biases ------------------------------------
            mska_sb = small.tile([P, P], BF16, tag="mska")
            nc.gpsimd.dma_start(out=mska_sb, in_=mska)
            iden_sb = small.tile([P, P], BF16, tag="iden")
            nc.gpsimd.dma_start(out=iden_sb, in_=iden)
            # touch Exp once at t=0 so the ACT table load (~1.3us) overlaps
            # the input DMA phase
            warm_sb = small.tile([1, 8], F32, tag="warm")
            nc.vector.memset(warm_sb, 1.0)
            nc.scalar.activation(warm_sb, warm_sb, EXP)
            bq_sb = small.tile([P, NPAIR], F32, tag="bq")
            nc.gpsimd.dma_start(out=bq_sb, in_=bq.rearrange("(c p) -> p c", p=P))
            bk_sb = small.tile([P, NPAIR], F32, tag="bk")
            nc.gpsimd.dma_start(out=bk_sb, in_=bk.rearrange("(c p) -> p c", p=P))
            bv_sb = small.tile([1, HK], BF16, tag="bv")
            nc.gpsimd.dma_start(out=bv_sb, in_=bv.rearrange("(o k) -> o k", o=1))
            ones_row = small.tile([1, P], BF16, tag="ones")
            nc.vector.memset(ones_row, 1.0)

            # ---- input DMAs (issue order = need order) ------------------
            # x chunks qc-major so the first projections can start early
            xt = [big.tile([P, S], BF16, tag=f"xt{dt}", name=f"xt{dt}") for dt in range(NDT)]
            wq_sb = []
            wk_sb = []
            for dram, lst, nm, eng in ((wq, wq_sb, "wq", nc.sync),
                                       (wk, wk_sb, "wk", nc.gpsimd)):
                for dt in range(NDT):
                    t = wts.tile([P, HK], BF16, tag=f"{nm}{dt}")
                    eng.dma_start(out=t, in_=dram[dt * P:(dt + 1) * P, :])
                    lst.append(t)
            for qc in range(NQC):
                for dt in range(NDT):
                    eng = nc.scalar if dt % 2 == 0 else nc.gpsimd
                    eng.dma_start(
                        out=xt[dt][:, qc * QW:(qc + 1) * QW],
                        in_=xT[dt * P:(dt + 1) * P, qc * QW:(qc + 1) * QW],
                    )
            wv_sb = []
            for dt in range(NDT):
                t = wts.tile([P, HK], BF16, tag=f"wv{dt}")
                nc.sync.dma_start(out=t, in_=wv[dt * P:(dt + 1) * P, :])
                wv_sb.append(t)
            wo_sb = []
            for c in range(NPAIR):
                t = wts.tile([P, D], BF16, tag=f"wo{c}")
                nc.scalar.dma_start(out=t, in_=wo[c * P:(c + 1) * P, :])
                wo_sb.append(t)

            # ---- persistent compute tiles -------------------------------
            QT = [big.tile([P, S], BF16, tag=f"qt{c}", name=f"qt{c}") for c in range(NPAIR)]
            KT = [big.tile([P, S], BF16, tag=f"kt{c}", name=f"kt{c}") for c in range(NPAIR)]
            ZT = [big.tile([P, S], BF16, tag=f"zt{c}", name=f"zt{c}") for c in range(NPAIR)]
            Vt = []
            for st in range(NST):
                t = vpool.tile([P, HL * VTW], BF16, tag=f"v{st}", name=f"v{st}")
                vv = t.rearrange("p (h c) -> p h c", c=VTW)
                nc.vector.memset(vv[:, :, 0:1], 1.0)
                nc.vector.memset(vv[:, :, 1:DH], 0.0)
                Vt.append(t)

            # ---- interleaver state --------------------------------------
            state = {"pe": 0.0, "act": 0.0, "copy_alt": 0}
            fillers = []      # list of (emit_fn, pe_cost)

            def copy_engine():
                # alternate proj-copy engine between ACT and DVE to split
                # the PSUM-egress load
                state["copy_alt"] += 1
                return nc.scalar if state["copy_alt"] % 2 == 0 else nc.vector

            def emit_qk(which, pr, qc):
                w_sb, b_sb, dst = (
                    (wq_sb, bq_sb, QT) if which == "q" else (wk_sb, bk_sb, KT)
                )
                pt = ps_p.tile([P, QW], F32, tag="psp")
                for dt in range(NDT):
                    nc.tensor.matmul(
                        pt,
                        lhsT=w_sb[dt][:, pr * P:(pr + 1) * P],
                        rhs=xt[dt][:, qc * QW:(qc + 1) * QW],
                        start=(dt == 0),
                        stop=(dt == NDT - 1),
                    )
                eng = copy_engine()
                dst_ap = dst[pr][:, qc * QW:(qc + 1) * QW]
                if eng is nc.scalar:
                    nc.scalar.add(dst_ap, pt, b_sb[:, pr:pr + 1])
                else:
                    nc.vector.tensor_scalar_add(dst_ap, pt, b_sb[:, pr:pr + 1])
                state["pe"] += 6 * QW * PEC

            def emit_v(st):
                pt = ps_p.tile([P, QW], F32, tag="psp")
                for dt in range(NDT):
                    nc.tensor.matmul(
                        pt[:, 0:HK],
                        lhsT=xt[dt][:, st * P:(st + 1) * P],
                        rhs=wv_sb[dt],
                        start=(dt == 0),
                        stop=False,
                    )
                nc.tensor.matmul(
                    pt[:, 0:HK],
                    lhsT=ones_row,
                    rhs=bv_sb,
                    start=False,
                    stop=True,
                )
                vv = Vt[st].rearrange("p (h c) -> p h c", c=VTW)
                eng = copy_engine()
                eng_copy = nc.scalar.copy if eng is nc.scalar else nc.vector.tensor_copy
                eng_copy(
                    vv[:, :, DH:VTW],
                    pt[:, 0:HK].rearrange("p (h c) -> p h c", c=DH),
                )
                state["pe"] += 7 * HK * PEC

            def emit_oproj(qc, dt):
                pt = ps_p.tile([P, QW], F32, tag="psp")
                for c in range(NPAIR):
                    nc.tensor.matmul(
                        pt,
                        lhsT=wo_sb[c][:, dt * P:(dt + 1) * P],
                        rhs=ZT[c][:, qc * QW:(qc + 1) * QW],
                        start=(c == 0),
                        stop=(c == NPAIR - 1),
                    )
                osb = otp.tile([P, QW], F32, tag="ot")
                eng = copy_engine()
                eng_copy = nc.scalar.copy if eng is nc.scalar else nc.vector.tensor_copy
                eng_copy(osb, pt)
                state["dma_alt"] = state.get("dma_alt", 0) + 1
                dma_eng = nc.sync if state["dma_alt"] % 2 == 0 else nc.gpsimd
                dma_eng.dma_start(
                    out=out[dt * P:(dt + 1) * P, qc * QW:(qc + 1) * QW],
                    in_=osb,
                )
                state["pe"] += NPAIR * QW * PEC

            pending = []      # (ready_step, emit_fn) gated fillers
            state["step"] = 0

            def pull_fillers():
                while pending and pending[0][0] <= state["step"]:
                    fillers.append(pending.pop(0)[1])
                while fillers and state["act"] > state["pe"]:
                    fn = fillers.pop(0)
                    fn()

            # dependency bookkeeping: JIT-emit projections a chain needs
            qk_done = set()
            v_done = set()

            def need_qk(pr, qc):
                for q in range(qc + 1):
                    for which in ("q", "k"):
                        if (which, pr, q) not in qk_done:
                            qk_done.add((which, pr, q))
                            emit_qk(which, pr, q)

            def need_v(kt_max):
                for st in range(kt_max + 1):
                    if st not in v_done:
                        v_done.add(st)
                        emit_v(st)

            # ---- attention chains ---------------------------------------
            def emit_chain(pr, qc):
                nkt = 4 * qc + 4
                need_qk(pr, qc)
                need_v(nkt - 1)
                q0 = qc * QW
                O_ab = [ps_o.tile([P, QW], F32, tag="o", name=f"o{i}") for i in range(2)]
                for kt in range(nkt):
                    o = max(0, P * kt - q0)
                    diag = P * kt >= q0
                    w = QW - o
                    sp = ps_s.tile([P, 2 * QW], F32, tag="s")
                    for hh in range(2):
                        lo = hh * DH
                        nc.tensor.matmul(
                            sp[:, hh * QW + o:hh * QW + QW],
                            lhsT=KT[pr][lo:lo + DH, kt * P:(kt + 1) * P],
                            rhs=QT[pr][lo:lo + DH, q0 + o:q0 + QW],
                            start=True,
                            stop=not diag,
                        )
                    if diag:
                        for hh in range(2):
                            nc.tensor.matmul(
                                sp[:, hh * QW + o:hh * QW + o + P],
                                lhsT=iden_sb,
                                rhs=mska_sb,
                                start=False,
                                stop=True,
                            )
                    # one contiguous exp for both heads; the junk columns
                    # [QW:QW+o) in the middle are never read downstream
                    pt = pp.tile([P, 2 * QW], BF16, tag="p")
                    nc.scalar.activation(
                        pt[:, o:2 * QW], sp[:, o:2 * QW], EXP, scale=SCALE
                    )
                    for hh in range(2):
                        h0 = (2 * pr + hh) * VTW
                        nc.tensor.matmul(
                            O_ab[hh][:, o:QW],
                            lhsT=Vt[kt][:, h0:h0 + VTW],
                            rhs=pt[:, hh * QW + o:hh * QW + QW],
                            start=(kt == 0),
                            stop=(kt == nkt - 1),
                        )
                    state["pe"] += (3 * w + (2 * P if diag else 0)) * PEC
                    state["act"] += (2 * QW - o) * ACTC + ACT_OVH
                    state["step"] += 1
                    pull_fillers()
                # normalize: ZT = O[0:64] * 1/sums, sums = O[64]
                for hh in range(2):
                    lo = hh * DH
                    rr = rr_pool.tile([1, QW], F32, tag="rr")
                    nc.vector.reciprocal_approx_fast(out=rr, in_=O_ab[hh][0:1, :])
                    rb = rb_pool.tile([P, QW], F32, tag="rb")
                    nc.gpsimd.partition_broadcast(rb, rr)
                    nc.vector.tensor_mul(
                        ZT[pr][lo:lo + DH, q0:q0 + QW], O_ab[hh][DH:P, :],
                        rb[DH:P, :]
                    )

            # o-proj units for qc become pullable a few attention steps
            # after chain (2, qc)'s normalize was emitted
            for pr, qc in CHAIN_ORDER:
                emit_chain(pr, qc)
                if pr == 2:
                    for dt in range(NDT):
                        pending.append(
                            (state["step"] + 4,
                             lambda qc=qc, dt=dt: emit_oproj(qc, dt))
                        )

            # drain remaining fillers (final o-proj blocks)
            for _, fn in pending:
                fillers.append(fn)
            for fn in fillers:
                fn()

    nc.compile()
    return nc


_NC_CACHE = {}


def _get_nc():
    if "nc" not in _NC_CACHE:
        _NC_CACHE["nc"] = build_nc()
    return _NC_CACHE["nc"]


def make_in_maps(x, W_Q, W_K, W_V, W_O, b_Q, b_K, b_V, b_O):
    mask_add = np.tril(np.full((P, P), -1e4, np.float32), k=-1)
    identity = np.eye(P, dtype=np.float32)
    in_maps = []
    for c in range(8):
        b, hh = divmod(c, 2)
        hs = slice(HL * hh, HL * hh + HL)
        in_maps.append({
            "xT": _np_in(np.asarray(x[b]).T),
            "wq": _np_in(np.asarray(W_Q[hs]).transpose(1, 0, 2).reshape(D, HK)),
            "wk": _np_in(np.asarray(W_K[hs]).transpose(1, 0, 2).reshape(D, HK)),
            "wv": _np_in(np.asarray(W_V[hs]).transpose(1, 0, 2).reshape(D, HK)),
            "wo": _np_in(np.asarray(W_O[hs]).reshape(HK, D)),
            "bq": np.ascontiguousarray(np.asarray(b_Q[hs]).reshape(HK), np.float32),
            "bk": np.ascontiguousarray(np.asarray(b_K[hs]).reshape(HK), np.float32),
            "bv": _np_in(np.asarray(b_V[hs]).reshape(HK)),
            "mska": _np_in(mask_add),
            "iden": _np_in(identity),
        })
    return in_maps


def run(inputs, trace=False):
    nc = _get_nc()
    in_maps = make_in_maps(**inputs)
    res = bass_utils.run_bass_kernel_spmd(
        nc, in_maps, core_ids=list(range(8)), trace=trace,
        **({"trace_cores": [0]} if trace else {}),
    )
    outs = [r["out"] for r in res.results]
    bo = np.ascontiguousarray(np.asarray(inputs["b_O"]), np.float32)
    full = np.empty((B, S, D), np.float32)
    for b in range(B):
        full[b] = (outs[2 * b] + outs[2 * b + 1]).T + bo
    return full, res


def kernel(**inputs):
    full, _ = run(inputs)
    return full
